# revision 20
# baseline (speedup 1.0000x reference)
"""ConvLinformer forward on 8 Trainium2 NeuronCores (Bass/Tile).

Sharding: 8-way over sequence (512 tokens/core/batch). Dense weights
replicated (pre-cast to bf16 on host); conv kernels channel-sliced per core
and pre-transposed on host to [c, s, o].

Linformer layers use a z-first formulation: z = pk^T @ xn (tiny, per-batch)
is AllReduced, then k_ = Wk^T z and v_ = z_v Wv are computed locally from
the reduced z. One merged AllReduce per (layer, batch) carries both k and v.
Conv layers keep the per-batch AllToAll of ke/ve channel slices, with one
merged AllReduce of the k_/v_ partials per batch.

The whole layer is organized as two per-batch streams; batch 1's FFN w2 pass
of the previous layer is emitted between batch 0's collective issue and its
first use, so collectives hide under independent PE work.

Token order within each core's 512-token shard is permuted on the host to
(s, w) order (n = w*16 + s) so the conv's stride-16 window gather becomes
contiguous; the host un-permutes the output.

Layout: residual stream feature-major in SBUF: x^T = [128, (dt:8, b:2,
tl:512)] f32; LN output / activations bf16; matmuls bf16 (f32r for LN
statistics and broadcast outer-products).
"""

import os
from contextlib import ExitStack

import numpy as np

import concourse.bacc as bacc
import concourse.mybir as mybir
import concourse.tile as tile
from concourse.bass_utils import run_bass_kernel_spmd
from concourse.masks import make_identity

P = 128
B, N, D, H, DH, K, S, DFF, L = 2, 4096, 1024, 8, 128, 256, 16, 4096, 2
NC = 8
NL = N // NC          # 512 local tokens per batch
T = B * NL            # 1024 local tokens, free layout (b, tl)
DT = D // P           # 8 feature tiles
DFT = DFF // P        # 32 dff tiles
KT = K // P           # 2 kv-position tiles
W = NL // S           # 32 conv windows per core per batch
SCALE = float(DH) ** -0.5

F32 = mybir.dt.float32
F32R = mybir.dt.float32r
BF16 = mybir.dt.bfloat16
AX = mybir.AxisListType
OP = mybir.AluOpType
AF = mybir.ActivationFunctionType

F8 = mybir.dt.float8e4
NP_BF16 = mybir.dt.np(BF16)
NP_F8 = mybir.dt.np(F8)
W8SCALE = 32.0           # host pre-scale on fp8 FFN weights
DR = mybir.MatmulPerfMode.DoubleRow

# token permutation: tl' = s*W + w  <->  n_local = w*S + s
_TL = np.arange(NL)
PERM = (_TL % W) * S + _TL // W      # n_local for each permuted slot tl'


VEC_NAMES = ["ln1_g", "ln1_b", "bo", "ln2_g", "ln2_b", "b2"]

LAYERS = [("lin", 0), ("lin", 1), ("conv", 0), ("conv", 1)]


def _declare_io(nc):
    """Inputs packed into a few big tensors (layer index: lin0,lin1,conv0,conv1)
    to keep the jit arg count (and client dispatch cost) small."""
    d = {}
    d["x_local"] = nc.dram_tensor("x_local", [B, NL, D], F32, kind="ExternalInput").ap()
    wqkvo = nc.dram_tensor("wqkvo", [2 * L, 4, D, D], BF16, kind="ExternalInput").ap()
    w1a = nc.dram_tensor("w1a", [2 * L, D, DFF], BF16, kind="ExternalInput").ap()
    w2a = nc.dram_tensor("w2a", [2 * L, DFF, D], BF16, kind="ExternalInput").ap()
    # pkv: token-major rows (PERM'd local tokens), cols = [pk | pv]
    pkv = nc.dram_tensor("pkv", [L, NL, 2 * K], BF16, kind="ExternalInput").ap()
    wtc = nc.dram_tensor("wtc", [L, 2, P, S * D], BF16, kind="ExternalInput").ap()
    vecs = nc.dram_tensor("vecs", [2 * L, 6, D], F32, kind="ExternalInput").ap()
    b1a = nc.dram_tensor("b1a", [2 * L, DFF], F32, kind="ExternalInput").ap()
    for kind in ("lin", "conv"):
        for li in range(L):
            pre = f"{kind}{li}_"
            idx = (0 if kind == "lin" else L) + li
            for vi, v in enumerate(VEC_NAMES):
                d[pre + v] = vecs[idx, vi]
            d[pre + "b1"] = b1a[idx]
            for wi, w in enumerate(("wq", "wk", "wv", "wo")):
                d[pre + w] = wqkvo[idx, wi]
            d[pre + "w1"] = w1a[idx]
            d[pre + "w2"] = w2a[idx]
            if kind == "lin":
                d[pre + "pkv"] = pkv[li]
            else:
                d[pre + "pk"] = wtc[li, 0]
                d[pre + "pv"] = wtc[li, 1]
    d["y_local"] = nc.dram_tensor("y_local", [B, NL, D], F32, kind="ExternalOutput").ap()
    return d


NO_CC = bool(os.environ.get("KERNEL_NO_CC"))


def _collective(nc, kind, op, ins, outs):
    """Real collective, or (timing-probe only) a local DMA of equal size."""
    if NO_CC:
        nc.sync.dma_start(outs[0], ins[0])
    else:
        nc.gpsimd.collective_compute(kind, op, replica_groups=[list(range(NC))],
                                     ins=ins, outs=outs)


class Ctx:
    def __init__(self, nc, tc, io):
        self.nc, self.tc, self.io = nc, tc, io


def _load_col(ctx, dram_vec, width, pool, name):
    """Load a [width*128] dram vector as a [128, width] column tile (f32)."""
    nc = ctx.nc
    t = pool.tile([P, width], F32, name=name)
    nc.sync.dma_start(t[:], dram_vec.rearrange("(w p) -> p w", p=P))
    return t


def _load_row(ctx, dram_vec, pool, name):
    """Load a [D] dram vector as a [1, D] row tile (f32r)."""
    nc = ctx.nc
    t = pool.tile([1, D], F32R, name=name)
    nc.sync.dma_start(t[:], dram_vec.rearrange("(o d) -> o d", o=1).bitcast(F32R))
    return t


def _layernorm_b(ctx, x, grow, bcol, xn_sl, c, sb, xqp):
    """xn[:, :, c] = LN(x[:, :, c])*g + b for one 512-token batch chunk c.

    Stats via f32r ones-matmuls; per-token scale/offset broadcast to
    [128,512] via PE outer products: B1 = g (x) r,  B2 = g (x) s0.
    xn = x*B1 + B2 + b.
    """
    nc = ctx.nc
    st1 = ctx.ps_row.tile([1, 512], F32, tag="tp", name="st1")
    st2 = ctx.ps_row.tile([1, 512], F32, tag="tp", name="st2")
    for dt in range(DT):
        xc = x[:, dt * T + c * 512:][:, :512]
        nc.tensor.matmul(st1[:], ctx.ones_col[:], xc,
                         start=(dt == 0), stop=(dt == DT - 1))
    for dt in range(DT):
        xc = x[:, dt * T + c * 512:][:, :512]
        xsq = xqp.tile([P, 512], F32R, tag="xsq")
        nc.vector.tensor_mul(xsq[:], xc.bitcast(F32), xc.bitcast(F32))
        nc.tensor.matmul(st2[:], ctx.ones_col[:], xsq[:],
                         start=(dt == 0), stop=(dt == DT - 1))
    m_row = sb.tile([1, 512], F32, tag="mrow")
    nc.vector.tensor_scalar_mul(m_row[:], st1[:], 1.0 / D)
    msq = sb.tile([1, 512], F32, tag="msq")
    nc.vector.tensor_mul(msq[:], m_row[:], m_row[:])
    var = sb.tile([1, 512], F32, tag="var")
    nc.vector.scalar_tensor_tensor(var[:], st2[:], 1.0 / D, msq[:], OP.mult, OP.subtract)
    sd = sb.tile([1, 512], F32, tag="sd")
    nc.scalar.activation(sd[:], var[:], AF.Sqrt, bias=ctx.eps_b[:], scale=1.0)
    r_row = sb.tile([1, 512], F32R, tag="rrow")
    with nc.allow_low_precision(reason="fp22 is plenty for 1/std"):
        nc.vector.reciprocal(r_row[:], sd[:])
    s0_row = sb.tile([1, 512], F32R, tag="s0row")
    nc.vector.scalar_tensor_tensor(s0_row[:], m_row[:], -1.0,
                                   r_row[:].bitcast(F32), OP.mult, OP.mult)
    for dt in range(DT):
        b1bc = ctx.ps_row.tile([P, 512], F32, tag="tp", name="b1bc")
        nc.tensor.matmul(b1bc[:], grow[:, dt * P:(dt + 1) * P],
                         r_row[:], start=True, stop=True)
        b2bc = ctx.ps_row.tile([P, 512], F32, tag="tp", name="b2bc")
        nc.tensor.matmul(b2bc[:], grow[:, dt * P:(dt + 1) * P],
                         s0_row[:], start=True, stop=True)
        sl = slice(dt * T + c * 512, dt * T + c * 512 + 512)
        t1 = sb.tile([P, 512], BF16, tag="t1")
        nc.vector.tensor_mul(t1[:], x[:, sl].bitcast(F32), b1bc[:])
        nc.vector.scalar_tensor_tensor(xn_sl(dt), t1[:],
                                       bcol[:, dt:dt + 1], b2bc[:],
                                       OP.add, OP.add)


def _load_w(ctx, w_dram, pool, name):
    """Load a [D, D] bf16 weight as one [128, DT*D] tile (dt-major)."""
    nc = ctx.nc
    t = pool.tile([P, DT * D], BF16, tag=name, name=name)
    nc.sync.dma_start(t[:].rearrange("p (dt o) -> p dt o", dt=DT),
                      w_dram.rearrange("(dt p) o -> p dt o", p=P))
    return t


def _proj_b(ctx, w_sb, src, b, out_cb):
    """Feature-major projection for one batch: psum[ot] = sum_dt W.T @ src."""
    nc = ctx.nc
    for ot in range(DT):
        pp = ctx.ps_a.tile([P, 512], F32, tag="mm", name="pp")
        for dt in range(DT):
            nc.tensor.matmul(pp[:], w_sb[:, dt * D + ot * P:][:, :P],
                             src[:, dt * T + b * 512:][:, :512],
                             start=(dt == 0), stop=(dt == DT - 1))
        out_cb(ot, pp)


def _lin_z(ctx, li, b, xn, tkp, zsp):
    """Transpose xn(b) to token-major, compute z = pk^T xn (k|v interleaved),
    stage to DRAM, and issue the merged AllReduce for batch b."""
    nc, tc = ctx.nc, ctx.tc
    # transpose xn(b) -> xn_tok [128 tok, (nt, 1024 feat)]
    xt = tkp.tile([P, 4 * D], BF16, tag="xtok", name=f"xtok{b}")
    for nt in range(4):
        for dg in range(2):
            tps = ctx.ps_c.tile([P, 512], BF16, tag="tp")
            for i in range(4):
                dt = dg * 4 + i
                nc.tensor.transpose(
                    tps[:, i * P:(i + 1) * P],
                    xn[:, dt * T + b * 512 + nt * P:][:, :P], ctx.ident_b[:])
            nc.vector.tensor_copy(
                xt[:].rearrange("p (nt dt f) -> p nt dt f", nt=4, dt=DT)[
                    :, nt, dg * 4:(dg + 1) * 4],
                tps[:].rearrange("p (i f) -> p i f", i=4))
    # z[dt] = sum_nt xt[nt, dt-chunk].T @ pkv[nt]  -> [128 c, 512 (k|v)]
    zst = zsp.tile([P, DT * 512], BF16, tag="zst", name=f"zst{b}")
    for dt in range(DT):
        zp = ctx.ps_a.tile([P, 512], F32, tag="mm", name="zp")
        for nt in range(4):
            nc.tensor.matmul(zp[:], xt[:, nt * D + dt * P:][:, :P],
                             ctx.pkv_sb[:, nt * 2 * K:][:, :2 * K],
                             start=(nt == 0), stop=(nt == 3))
        nc.vector.tensor_copy(zst[:, dt * 512:][:, :512], zp[:])
    nc.sync.dma_start(ctx.cc_in[b][:], zst[:])
    _collective(nc, "AllReduce", OP.add,
                ins=[ctx.cc_in[b][:]], outs=[ctx.cc_out[b][:]])


def _lin_kp(ctx, li, b, pre, kcp, zp_, kvp):
    """Post-AllReduce: k_ = Wk^T z_k (feature-major), v_ = z_v Wv
    (token-major); Wk/Wv streamed in 2KB chunks."""
    nc = ctx.nc
    io = ctx.io
    zar = zp_.tile([P, DT * 512], BF16, tag="zar", name="zar")
    nc.sync.dma_start(zar[:], ctx.cc_out[b][:])
    kvk = kvp.tile([P, 2048], BF16, tag="kvk", name="kvk")
    kvv = kvp.tile([P, 2048], BF16, tag="kvv", name="kvv")
    wk_v = io[pre + "wk"].rearrange("(dt p) o -> p dt o", p=P)
    wv_v = io[pre + "wv"].rearrange("(dt p) o -> p dt o", p=P)
    for ot in range(DT):
        wkc = kcp.tile([P, D], BF16, tag="wkc", name="wkc")
        nc.sync.dma_start(wkc[:].rearrange("p (dt o) -> p dt o", dt=DT),
                          wk_v[:, :, ot * P:(ot + 1) * P])
        kp = ctx.ps_a.tile([P, K], F32, tag="mm", name="kp")
        for dt in range(DT):
            nc.tensor.matmul(kp[:], wkc[:, dt * P:(dt + 1) * P],
                             zar[:, dt * 512:][:, :K],
                             start=(dt == 0), stop=(dt == DT - 1))
        nc.vector.tensor_copy(kvk[:, ot * K:][:, :K], kp[:])
    vps = [ctx.ps_a.tile([P, 512], F32, tag="mm", name=f"vp{i}")
           for i in range(4)]
    for dt in range(DT):
        wvc = kcp.tile([P, D], BF16, tag="wkc", name="wvc")
        nc.sync.dma_start(wvc[:], wv_v[:, dt])
        for kt in range(KT):
            lhs = zar[:, dt * 512 + K + kt * P:][:, :P]
            for c2 in range(2):
                nc.tensor.matmul(vps[kt * 2 + c2][:], lhs,
                                 wvc[:, c2 * 512:][:, :512],
                                 start=(dt == 0), stop=(dt == DT - 1))
    for kt in range(KT):
        for c2 in range(2):
            nc.vector.tensor_copy(kvv[:, kt * D + c2 * 512:][:, :512],
                                  vps[kt * 2 + c2][:])
    return kvk, kvv


def _conv_stage(ctx, li, b, xn, wk_sb, wv_sb, ksp):
    """Compute ke/ve for batch b's local tokens, stage, issue AllToAll."""
    nc = ctx.nc
    for ten, w_sb in ((0, wk_sb), (1, wv_sb)):
        for ot in range(DT):
            pp = ctx.ps_a.tile([P, 512], F32, tag="mm", name="pp")
            for dt in range(DT):
                nc.tensor.matmul(pp[:], w_sb[:, dt * D + ot * P:][:, :P],
                                 xn[:, dt * T + b * 512:][:, :512],
                                 start=(dt == 0), stop=(dt == DT - 1))
            st = ksp.tile([P, 512], BF16, tag="kest", name="kest")
            nc.vector.tensor_copy(st[:], pp[:])
            nc.sync.dma_start(ctx.a2a_in[b][ot, ten], st[:])
    _collective(nc, "AllToAll", OP.bypass,
                ins=[ctx.a2a_in[b][:]], outs=[ctx.a2a_out[b][:]])


def _conv_contract(ctx, li, b, pre, csp, arp, wcp):
    """Regroup the A2A'd ke/ve to s-major with GpSimd, contract with the conv
    kernel slice (streamed per-s 2KB chunks); stage merged k|v partials and
    issue one AllReduce for batch b."""
    nc = ctx.nc
    io = ctx.io
    arh = arp.tile([P, 4096], BF16, tag="arh", name="arh")
    for ten, wnm in ((0, "pk"), (1, "pv")):
        ecs_r = csp.tile([P, N], BF16, tag="kecs", name="ecs_r")
        nc.sync.dma_start(
            ecs_r[:].rearrange("c (j t) -> c j t", j=NC),
            ctx.a2a_out[b][:, ten].rearrange("j c t -> c j t"))
        # regroup peer-major (j, s, w) -> s-major (s, j, w) so the
        # conv matmul operands are contiguous per s (idle GpSimd)
        ecs = csp.tile([P, N], BF16, tag="kesg", name="ecs")
        for sh in range(2):  # halves: contraction starts on half 0
            nc.gpsimd.tensor_copy(
                ecs[:].rearrange("c (s j w) -> c s j w", s=S, j=NC)[
                    :, sh * (S // 2):(sh + 1) * (S // 2)],
                ecs_r[:].rearrange("c (j s w) -> c s j w", j=NC, s=S)[
                    :, sh * (S // 2):(sh + 1) * (S // 2)])
        if ten == 0:
            # k_^T feature-major: arh[:, ot*K : +K], cols (j, w) = t
            for og in range(2):
                kps = [ctx.ps_a.tile([P, K], F32, tag="mm", name=f"kp{i}")
                       for i in range(4)]
                for s in range(S):
                    wch = wcp.tile([P, D], BF16, tag="wtc", name="wch")
                    nc.sync.dma_start(wch[:], io[pre + wnm][:, s * D:(s + 1) * D])
                    for oi in range(4):
                        ot = og * 4 + oi
                        nc.tensor.matmul(kps[oi][:],
                                         wch[:, ot * P:(ot + 1) * P],
                                         ecs[:, s * K:][:, :K],
                                         start=(s == 0), stop=(s == S - 1))
                for oi in range(4):
                    nc.vector.tensor_copy(
                        arh[:, (og * 4 + oi) * K:][:, :K], kps[oi][:])
        else:
            # v_ token-major: arh[:, 2048 + kt*D + c2*512]
            vps = [ctx.ps_a.tile([P, 512], F32, tag="mm", name=f"vp{i}")
                   for i in range(4)]
            for s in range(S):
                wch = wcp.tile([P, D], BF16, tag="wtc", name="wch")
                nc.sync.dma_start(wch[:], io[pre + wnm][:, s * D:(s + 1) * D])
                for kt in range(KT):
                    lhs = ecs[:, s * K + kt * P:][:, :P]
                    for c2 in range(2):
                        nc.tensor.matmul(vps[kt * 2 + c2][:], lhs,
                                         wch[:, c2 * 512:][:, :512],
                                         start=(s == 0), stop=(s == S - 1))
            for kt in range(KT):
                for c2 in range(2):
                    nc.vector.tensor_copy(
                        arh[:, 2048 + kt * D + c2 * 512:][:, :512],
                        vps[kt * 2 + c2][:])
    nc.sync.dma_start(ctx.cc_in[b][:], arh[:])
    _collective(nc, "AllReduce", OP.add,
                ins=[ctx.cc_in[b][:]], outs=[ctx.cc_out[b][:]])


def _attention(ctx, b, qo_sb, kvk_sb, kvv_sb, pfx):
    """Per head h of batch b: scoresT -> exp -> transpose -> o; normalization
    by 1/sum folded into the output copy. o overwrites q's slice."""
    nc, tc = ctx.nc, ctx.tc
    with (
        tc.tile_pool(name=pfx + "sb", bufs=3) as sb,
        tc.tile_pool(name=pfx + "at", bufs=2) as atp,
    ):
        for h in range(H):
            at_sb = atp.tile([P, KT * 512], BF16, tag="at", name="at_sb")
            for tt in range(4):
                sc = ctx.ps_b.tile([P, K], F32, tag="att", name="sc")
                nc.tensor.matmul(sc[:],
                                 qo_sb[:, h * T + b * 512 + tt * P:][:, :P],
                                 kvk_sb[:, h * K:][:, :K],
                                 start=True, stop=True)
                a_e = sb.tile([P, K], BF16, tag="ae")
                den = sb.tile([P, 1], F32, tag="den")
                nc.scalar.activation(a_e[:], sc[:], AF.Exp, scale=SCALE, accum_out=den[:])
                inv = sb.tile([P, 1], F32, tag="inv")
                nc.vector.reciprocal(inv[:], den[:])
                a_r = sb.tile([P, K], BF16, tag="ar")
                nc.vector.tensor_scalar_mul(a_r[:], a_e[:], inv[:])
                tp = ctx.ps_c.tile([P, K], BF16, tag="tp")
                for kt in range(KT):
                    nc.tensor.transpose(tp[:, kt * P:(kt + 1) * P],
                                        a_r[:, kt * P:(kt + 1) * P], ctx.ident_b[:])
                nc.vector.tensor_copy(
                    at_sb[:].rearrange("p (kt t) -> p kt t", kt=KT)[:, :, tt * P:(tt + 1) * P],
                    tp[:].rearrange("p (kt t) -> p kt t", kt=KT))
            oo = ctx.ps_b.tile([P, 512], F32, tag="att", name="oo")
            for kt in range(KT):
                nc.tensor.matmul(oo[:],
                                 kvv_sb[:, kt * D + h * P:][:, :P],
                                 at_sb[:, kt * 512:][:, :512],
                                 start=(kt == 0), stop=(kt == KT - 1))
            nc.vector.tensor_copy(qo_sb[:, h * T + b * 512:][:, :512], oo[:])


def _ffn_w1(ctx, pre, xn2, b1_col, h_sb, w1p):
    """h = gelu(xn2 @ w1 + b1), streamed w1; both batches per weight load."""
    nc = ctx.nc
    io = ctx.io
    FG = 4            # ft tiles per streamed w1 chunk
    for fg in range(DFT // FG):
        w1g = w1p.tile([P, DT * FG * P], BF16, tag="w1g", name="w1g")
        nc.sync.dma_start(
            w1g[:].rearrange("p (dt f) -> p dt f", dt=DT),
            io[pre + "w1"].rearrange("(dt p) f -> p dt f", p=P)[
                :, :, fg * FG * P:(fg + 1) * FG * P])
        for fi in range(FG):
            ft = fg * FG + fi
            hh = [ctx.ps_a.tile([P, 512], F32, tag="mm", name="hh0"),
                  ctx.ps_a.tile([P, 512], F32, tag="mm", name="hh1")]
            for dt in range(DT):
                lhs = w1g[:, dt * FG * P + fi * P:][:, :P]
                for c in range(2):
                    nc.tensor.matmul(hh[c][:], lhs,
                                     xn2[:, dt * T + c * 512:][:, :512],
                                     start=(dt == 0), stop=(dt == DT - 1))
            for c in range(2):
                nc.scalar.activation(h_sb[:, ft * T + c * 512:][:, :512], hh[c][:],
                                     AF.Gelu, bias=b1_col[:, ft:ft + 1], scale=1.0)


def _ffn_w2(ctx, pre, x, b2_col, h_sb, w2p):
    """x += h @ w2 + b2, streamed w2; both batches per weight load."""
    nc = ctx.nc
    io = ctx.io
    for ot in range(DT):
        w2g = w2p.tile([P, DFT * P], BF16, tag="w2g", name="w2g")
        nc.sync.dma_start(
            w2g[:].rearrange("p (ft o) -> p ft o", ft=DFT),
            io[pre + "w2"].rearrange("(ft p) o -> p ft o", p=P)[:, :, ot * P:(ot + 1) * P])
        ff = [ctx.ps_a.tile([P, 512], F32, tag="mm", name="ff0"),
              ctx.ps_a.tile([P, 512], F32, tag="mm", name="ff1")]
        for ft in range(DFT):
            lhs = w2g[:, ft * P:][:, :P]
            for c in range(2):
                nc.tensor.matmul(ff[c][:], lhs, h_sb[:, ft * T + c * 512:][:, :512],
                                 start=(ft == 0), stop=(ft == DFT - 1))
        for c in range(2):
            sl = slice(ot * T + c * 512, ot * T + c * 512 + 512)
            nc.vector.scalar_tensor_tensor(x[:, sl], ff[c][:],
                                           b2_col[:, ot:ot + 1], x[:, sl].bitcast(F32),
                                           OP.add, OP.add)


class LayerEmitter:
    """Emits one layer's instruction stream in per-batch stages.

    Pool nesting (stack discipline): es (whole layer) > es_att (through
    attention, closed at ffn_w1(0)) > es_pro (prologue, closed at mid) /
    es_mid (mid-only, closed at attpost(0)). FFN pools enter es after
    es_att closes.
    """

    def __init__(self, ctx, kind, li):
        self.ctx, self.kind, self.li = ctx, kind, li
        self.pre = f"{kind}{li}_"
        self.pfx = self.pre
        self.es = ExitStack()        # closed at end of this layer's emission
        self.es_att = ExitStack()    # closed at ffn_w1(0)
        self.es_pro = ExitStack()    # closed at mid()
        self.es_mid = ExitStack()    # closed at attpost(0)

    def open_pools(self):
        ctx, tc = self.ctx, self.ctx.tc
        pfx = self.pfx
        # --- es: whole-layer pools ---
        self.dp = self.es.enter_context(
            tc.tile_pool(name=pfx + "dram", bufs=1, space="DRAM"))
        self.clp = self.es.enter_context(tc.tile_pool(name=pfx + "cl", bufs=1))
        self.wap = self.es.enter_context(tc.tile_pool(name=pfx + "wa", bufs=1))
        self.sbp = self.es.enter_context(tc.tile_pool(name=pfx + "sb", bufs=2))
        self.xqp = self.es.enter_context(tc.tile_pool(name=pfx + "xq", bufs=3))
        self.g1_row = _load_row(ctx, ctx.io[self.pre + "ln1_g"], self.clp, "g1r")
        self.b1n_col = _load_col(ctx, ctx.io[self.pre + "ln1_b"], DT, self.clp, "b1nc")
        self.bo_col = _load_col(ctx, ctx.io[self.pre + "bo"], DT, self.clp, "boc")
        self.b1_col = _load_col(ctx, ctx.io[self.pre + "b1"], DFT, self.clp, "b1c")
        self.b2_col = _load_col(ctx, ctx.io[self.pre + "b2"], DT, self.clp, "b2c")
        self.xn = self.wap.tile([P, DT * T], BF16, tag="workA", name="xn")

        ctx.cc_in = [self.dp.tile([P, 4096], BF16, tag=f"cc_in{b}", name=f"cc_in{b}")
                     for b in range(B)]
        ctx.cc_out = [self.dp.tile([P, 4096], BF16, tag=f"cc_out{b}", name=f"cc_out{b}",
                                   addr_space="Shared") for b in range(B)]
        # --- es_att: pools alive through attention / wo ---
        self.wqp = self.es_att.enter_context(tc.tile_pool(name=pfx + "wq", bufs=1))
        self.wq_sb = _load_w(ctx, ctx.io[self.pre + "wq"], self.wqp, "wq")
        self.wop = self.es_att.enter_context(tc.tile_pool(name=pfx + "wo", bufs=1))
        self.wo_sb = _load_w(ctx, ctx.io[self.pre + "wo"], self.wop, "wo")
        if self.kind == "lin":
            self.kcp = self.es_att.enter_context(
                tc.tile_pool(name=pfx + "kc", bufs=3))
            self.zp_ = self.es_att.enter_context(tc.tile_pool(name=pfx + "zp", bufs=1))
        self.kvp = self.es_att.enter_context(tc.tile_pool(name=pfx + "kv", bufs=1))
        self.qop = self.es_att.enter_context(tc.tile_pool(name=pfx + "qo", bufs=1))
        self.qo_sb = self.qop.tile([P, DT * T], BF16, tag="qo", name="qo_sb")
        # --- es_pro: prologue-only pools ---
        if self.kind == "lin":
            pp_ = self.es_pro.enter_context(tc.tile_pool(name=pfx + "pkv", bufs=1))
            ctx.pkv_sb = pp_.tile([P, 4 * 2 * K], BF16, name="pkv_sb")
            ctx.nc.sync.dma_start(
                ctx.pkv_sb[:].rearrange("p (nt kv) -> p nt kv", nt=4),
                ctx.io[self.pre + "pkv"].rearrange("(nt p) kv -> p nt kv", p=P))
            self.tkp = self.es_pro.enter_context(tc.tile_pool(name=pfx + "tk", bufs=1))
            self.zsp = self.es_pro.enter_context(tc.tile_pool(name=pfx + "zs", bufs=1))
        else:
            ctx.a2a_in = [self.dp.tile([NC, 2, P, NL], BF16, tag=f"a2a_in{b}",
                                       name=f"a2a_in{b}") for b in range(B)]
            ctx.a2a_out = [self.dp.tile([NC, 2, P, NL], BF16, tag=f"a2a_out{b}",
                                        name=f"a2a_out{b}") for b in range(B)]
            self.wkvp = self.es_pro.enter_context(
                tc.tile_pool(name=pfx + "wkv", bufs=1))
            self.ksp = self.es_pro.enter_context(tc.tile_pool(name=pfx + "kest", bufs=3))
            self.wk_sb = _load_w(ctx, ctx.io[self.pre + "wk"], self.wkvp, "wk")
            self.wv_sb = _load_w(ctx, ctx.io[self.pre + "wv"], self.wkvp, "wv")

    def prologue(self, b, x):
        ctx = self.ctx
        xn = self.xn
        _layernorm_b(ctx, x, self.g1_row, self.b1n_col,
                     lambda dt, b=b: xn[:, dt * T + b * 512:][:, :512], b,
                     self.sbp, self.xqp)
        if self.kind == "lin":
            _lin_z(ctx, self.li, b, self.xn, self.tkp, self.zsp)
        else:
            _conv_stage(ctx, self.li, b, self.xn, self.wk_sb, self.wv_sb, self.ksp)

    def mid(self, x):
        ctx, tc = self.ctx, self.ctx.tc
        self.es_pro.close()
        if self.kind == "conv":
            self.csp = self.es_mid.enter_context(
                tc.tile_pool(name=self.pfx + "cs", bufs=1))
            self.arp = self.es_mid.enter_context(
                tc.tile_pool(name=self.pfx + "ar", bufs=1))
            self.wcp = self.es_mid.enter_context(
                tc.tile_pool(name=self.pfx + "wc", bufs=3))
        nc = ctx.nc
        if self.kind == "conv":
            for b in range(B):
                def q_cb(ot, pp, b=b):
                    nc.vector.tensor_copy(
                        self.qo_sb[:, ot * T + b * 512:][:, :512], pp[:])
                _proj_b(ctx, self.wq_sb, self.xn, b, q_cb)
                _conv_contract(ctx, self.li, b, self.pre,
                               self.csp, self.arp, self.wcp)
        else:
            # both batches per weight load (consecutive MMs share lhsT)
            for ot in range(DT):
                pp = [ctx.ps_a.tile([P, 512], F32, tag="mm", name="qp0"),
                      ctx.ps_a.tile([P, 512], F32, tag="mm", name="qp1")]
                for dt in range(DT):
                    lhs = self.wq_sb[:, dt * D + ot * P:][:, :P]
                    for c in range(2):
                        nc.tensor.matmul(pp[c][:], lhs,
                                         self.xn[:, dt * T + c * 512:][:, :512],
                                         start=(dt == 0), stop=(dt == DT - 1))
                for c in range(2):
                    nc.vector.tensor_copy(
                        self.qo_sb[:, ot * T + c * 512:][:, :512], pp[c][:])
        self.g2_row = _load_row(ctx, ctx.io[self.pre + "ln2_g"], self.clp, "g2r")
        self.b2n_col = _load_col(ctx, ctx.io[self.pre + "ln2_b"], DT, self.clp, "b2nc")

    def attpost(self, b, x):
        if b == 0 and self.kind == "conv":
            self.es_mid.close()
        ctx = self.ctx
        nc = ctx.nc
        if self.kind == "lin":
            kvk, kvv = _lin_kp(ctx, self.li, b, self.pre, self.kcp,
                               self.zp_, self.kvp)
        else:
            kvk = self.kvp.tile([P, 2048], BF16, tag="kvk", name="kvk")
            nc.sync.dma_start(kvk[:], ctx.cc_out[b][:, :2048])
            kvv = self.kvp.tile([P, 2048], BF16, tag="kvv", name="kvv")
            nc.sync.dma_start(kvv[:], ctx.cc_out[b][:, 2048:])
        _attention(ctx, b, self.qo_sb, kvk, kvv, self.pfx + f"att{b}")

        def wo_cb(ot, pp, b=b):
            sl = slice(ot * T + b * 512, ot * T + b * 512 + 512)
            nc.vector.scalar_tensor_tensor(x[:, sl], pp[:],
                                           self.bo_col[:, ot:ot + 1],
                                           x[:, sl].bitcast(F32),
                                           OP.add, OP.add)
        _proj_b(ctx, self.wo_sb, self.qo_sb, b, wo_cb)
        xn = self.xn
        _layernorm_b(ctx, x, self.g2_row, self.b2n_col,
                     lambda dt, b=b: xn[:, dt * T + b * 512:][:, :512], b,
                     self.sbp, self.xqp)

    def ffn(self, x):
        ctx, tc = self.ctx, self.ctx.tc
        self.es_att.close()
        self.w1p = self.es.enter_context(
            tc.tile_pool(name=self.pfx + "w1", bufs=2))
        self.hp = self.es.enter_context(
            tc.tile_pool(name=self.pfx + "h", bufs=1))
        self.w2p = self.es.enter_context(
            tc.tile_pool(name=self.pfx + "w2", bufs=2))
        h_sb = self.hp.tile([P, DFT * T], BF16, tag="h", name="h_sb")
        _ffn_w1(ctx, self.pre, self.xn, self.b1_col, h_sb, self.w1p)
        _ffn_w2(ctx, self.pre, x, self.b2_col, h_sb, self.w2p)

    def close(self):
        self.es.close()


def build_program():
    nc = bacc.Bacc("TRN2", target_bir_lowering=False, debug=False, num_devices=NC)
    io = _declare_io(nc)
    with tile.TileContext(nc) as tc:
        with (
            tc.tile_pool(name="cst", bufs=1) as cst,
            tc.tile_pool(name="xp", bufs=1) as xp,
            tc.tile_pool(name="psa", bufs=4, space="PSUM") as ps_a,
            tc.tile_pool(name="psb", bufs=2, space="PSUM") as ps_b,
            tc.tile_pool(name="psc", bufs=2, space="PSUM") as ps_c,
        ):
            ctx = Ctx(nc, tc, io)
            ctx.ps_a, ctx.ps_b, ctx.ps_c = ps_a, ps_b, ps_c
            ctx.ps_row = ps_c
            ident_f = cst.tile([P, P], F32, name="ident_f")
            make_identity(nc, ident_f[:])
            ctx.ident_r = cst.tile([P, P], F32R, name="ident_r")
            nc.vector.tensor_copy(ctx.ident_r[:], ident_f[:])
            ctx.ident_b = cst.tile([P, P], BF16, name="ident_b")
            nc.vector.tensor_copy(ctx.ident_b[:], ident_f[:])
            oc_f = cst.tile([P, 1], F32, name="oc_f")
            nc.vector.memset(oc_f[:], 1.0)
            ctx.ones_col = cst.tile([P, 1], F32R, name="ones_col")
            nc.vector.tensor_copy(ctx.ones_col[:], oc_f[:])
            ctx.eps_b = cst.tile([1, 1], F32, name="eps_b")
            nc.vector.memset(ctx.eps_b[:], 1e-5)

            # load x -> feature-major x^T
            x = xp.tile([P, DT * T], F32R, name="x")
            with tc.tile_pool(name="iop", bufs=3) as iop:
                for tt in range(8):  # tt = b*4 + nt
                    b, nt = divmod(tt, 4)
                    xtok = iop.tile([P, D], F32R, tag="xtok")
                    nc.sync.dma_start(xtok[:], io["x_local"][b, nt * P:(nt + 1) * P, :].bitcast(F32R))
                    for dg in range(2):
                        tps = ps_c.tile([P, 512], F32R, tag="tp")
                        for i in range(4):
                            dt = dg * 4 + i
                            nc.tensor.transpose(tps[:, i * P:(i + 1) * P],
                                                xtok[:, dt * P:(dt + 1) * P], ctx.ident_r[:])
                        nc.vector.tensor_copy(
                            x[:].rearrange("p (dt t) -> p dt t", dt=DT)[:, dg * 4:(dg + 1) * 4,
                                                                        b * 512 + nt * P:][:, :, :P],
                            tps[:].rearrange("p (i t) -> p i t", i=4).bitcast(F32))

            for kind, li in LAYERS:
                em = LayerEmitter(ctx, kind, li)
                em.open_pools()
                em.prologue(0, x)
                em.prologue(1, x)
                em.mid(x)
                em.attpost(0, x)
                em.attpost(1, x)
                em.ffn(x)
                em.close()

            # write out: transpose back to token-major
            with tc.tile_pool(name="oop", bufs=3) as oop:
                for tt in range(8):
                    b, nt = divmod(tt, 4)
                    ytok = oop.tile([P, D], F32, tag="ytok")
                    for dg in range(2):
                        tps = ps_c.tile([P, 512], F32R, tag="tp")
                        for i in range(4):
                            dt = dg * 4 + i
                            nc.tensor.transpose(tps[:, i * P:(i + 1) * P],
                                                x[:, dt * T + b * 512 + nt * P:][:, :P],
                                                ctx.ident_r[:])
                        nc.vector.tensor_copy(ytok[:, dg * 512:(dg + 1) * 512], tps[:].bitcast(F32))
                    nc.sync.dma_start(io["y_local"][b, nt * P:(nt + 1) * P, :], ytok[:])
    nc.compile()
    return nc


_PROGRAM = None


def _get_program():
    global _PROGRAM
    if _PROGRAM is None:
        _PROGRAM = build_program()
    return _PROGRAM


def _make_in_maps(inputs):
    def g(kind, nm, li):
        return np.asarray(inputs[f"{kind}_{nm}"][li], dtype=np.float32)

    wqkvo = np.stack([
        np.stack([g(kind, w, li) for w in ("wq", "wk", "wv", "wo")])
        for kind in ("lin", "conv") for li in range(L)
    ]).astype(NP_BF16)
    w1a = np.stack([g(kind, "w1", li)
                    for kind in ("lin", "conv") for li in range(L)]).astype(NP_BF16)
    w2a = np.stack([g(kind, "w2", li)
                    for kind in ("lin", "conv") for li in range(L)]).astype(NP_BF16)
    vecs = np.stack([
        np.stack([g(kind, v, li) for v in VEC_NAMES])
        for kind in ("lin", "conv") for li in range(L)
    ]).astype(np.float32)
    b1a = np.stack([g(kind, "b1", li)
                    for kind in ("lin", "conv") for li in range(L)]).astype(np.float32)

    in_maps = []
    for c in range(NC):
        xs = np.asarray(inputs["x"][:, c * NL:(c + 1) * NL, :], dtype=np.float32)
        # pkv rows: this core's PERM'd local tokens; cols [pk | pv]
        pkv = np.stack([
            np.concatenate([g("lin", nm, li)[c * NL:(c + 1) * NL, :][PERM]
                            for nm in ("pk", "pv")], axis=1)
            for li in range(L)
        ]).astype(NP_BF16)
        wtc = np.stack([
            np.stack([np.ascontiguousarray(
                g("conv", nm, li)[:, c * P:(c + 1) * P, :].transpose(1, 2, 0)
            ).reshape(P, S * D) for nm in ("pk", "pv")])
            for li in range(L)
        ]).astype(NP_BF16)
        m = {
            "x_local": np.ascontiguousarray(xs[:, PERM, :]),
            "wqkvo": wqkvo, "w1a": w1a, "w2a": w2a,
            "pkv": pkv, "wtc": wtc, "vecs": vecs, "b1a": b1a,
        }
        in_maps.append(m)
    return in_maps


def kernel(**inputs):
    nc = _get_program()
    in_maps = _make_in_maps(inputs)
    res = run_bass_kernel_spmd(nc, in_maps, core_ids=list(range(NC)))
    out = np.empty((B, N, D), dtype=np.float32)
    for c in range(NC):
        out[:, c * NL + PERM, :] = res.results[c]["y_local"]
    return out.astype(np.float32)
